# revision 30
# baseline (speedup 1.0000x reference)
"""Tensor-parallel transformer block (attention + MLP with RMSNorm) on 8 TRN2 NeuronCores.

Single-scope software-pipelined design (v3, batched DMA):
  - TP-8: c_attn column-sharded by heads (2 heads/core), c_proj row-sharded,
    fc1/fc2 column-sharded (704/core), mlp proj row-sharded. All bf16.
  - One TileContext scope; per-chunk emission interleaves attn(t) with
    MLP(t-1) so the tile scheduler hides the AllReduce and keeps PE busy.
  - DMA instruction count minimized: host pre-tiles x-hat / x / exp(alibi) /
    weights into [*, 128, batch, 512] layouts so each DMA moves 2-4 tiles
    contiguously per partition (HWDGE queue service is ~0.6us/op and was
    pacing the pipeline).
  - rmsnorm1 folded into the host-supplied x-hat; 1/sqrt(hd) folded into the
    q/k weight columns; causal mask+alibi pre-exponentiated host-side so
    probs = exp(scores) * alibi_exp (DVE 2x bf16 multiply, no add chain).
  - MLP residual reconstructs x2 = xh2 * rms (sqb2 broadcast) to skip a
    third arout read pass.
  - PSUM (8 banks): qkv(1, also rms2-stats row) + scores(2, also recip
    broadcasts) + po(1) + den(1) + pj(1) + facc(2: fc1/fc2/mlp-proj/pb2).
  - AllReduce carries attn_partial + x/8 (bf16); ReduceScatter carries
    mlp_partial + x2/8 (bf16), C-sharded; host transposes/concats shards.
"""
import numpy as np
import ml_dtypes

import concourse.bass as bass
import concourse.tile as tile
from concourse import bacc, mybir
from concourse.bass_utils import run_bass_kernel_spmd

f32 = mybir.dt.float32
f32r = mybir.dt.float32r
bf16 = mybir.dt.bfloat16
FA = mybir.ActivationFunctionType
ALU = mybir.AluOpType

NCORES = 8
T = 2048
C = 2048
NH = 16
HD = 128
HL = NH // NCORES          # 2 local heads
CL = HL * HD               # 256 local attn dims
DFF = 5632
DL = DFF // NCORES         # 704 local ffn dims
EPS = 1e-5
ISQ = float(1.0 / np.sqrt(HD))
NEG = -30000.0
KC = C // 128              # 16 contraction chunks
KG = KC // 4               # 4 groups of 4 chunks
NT4 = T // 512             # 4 T-chunks of 512
FCO = [128] * 5 + [64]     # DL partition chunks (704 = 5*128 + 64)
RG = [list(range(NCORES))]


def build(reps=1, collectives=True):
    nc = bacc.Bacc("TRN2", target_bir_lowering=False, debug=False,
                   enable_asserts=False,
                   num_devices=NCORES if collectives else 1)

    xt = nc.dram_tensor("xt", [KG, NT4, 128, 4, 512], bf16,
                        kind="ExternalInput").ap()
    xh_d = nc.dram_tensor("xh_d", [KG, NT4, 128, 4, 512], bf16,
                          kind="ExternalInput").ap()
    wqkv = nc.dram_tensor("wqkv", [KG, 128, 4, 3 * CL], bf16,
                          kind="ExternalInput").ap()
    alibiT = nc.dram_tensor("alibiT", [HL, T // 512, NT4, 128, 4, 512],
                            bf16, kind="ExternalInput").ap()
    wproj = nc.dram_tensor("wproj", [CL, C], bf16, kind="ExternalInput").ap()
    wfc1 = nc.dram_tensor("wfc1", [KG, 128, 4, DL], bf16,
                          kind="ExternalInput").ap()
    wfc2 = nc.dram_tensor("wfc2", [KG, 128, 4, DL], bf16,
                          kind="ExternalInput").ap()
    wmp = nc.dram_tensor("wmp", [DL, C], bf16, kind="ExternalInput").ap()
    out = nc.dram_tensor("out", [C // NCORES, T], bf16,
                         kind="ExternalOutput").ap()

    from contextlib import ExitStack
    with tile.TileContext(nc) as tc:
        with ExitStack() as stack:
            pool = lambda *a, **k: stack.enter_context(tc.tile_pool(*a, **k))
            consts = pool(name="consts", bufs=1)
            wpool = pool(name="weights", bufs=1)
            pers = pool(name="persist", bufs=1)
            qtp = pool(name="qt", bufs=2)
            xh4p = pool(name="xh4", bufs=4)
            al4p = pool(name="al4", bufs=2)
            vtp = pool(name="vt", bufs=1)
            ssbp = pool(name="ssb", bufs=1)
            prp = pool(name="pr", bufs=3)
            ytp = pool(name="yt", bufs=2)
            xtdp = pool(name="xtd2", bufs=1)
            obp = pool(name="ob2g", bufs=2)
            x2sp = pool(name="x2s2", bufs=2)
            x2bp = pool(name="x2b2", bufs=2)
            xqp = pool(name="xq", bufs=1)
            xh2p = pool(name="xh2", bufs=16)
            asbp = pool(name="asb", bufs=1)
            gpool = pool(name="g", bufs=6)
            rbp = pool(name="rb", bufs=1)
            tmpp = pool(name="tmp", bufs=1)
            smalls = pool(name="smalls", bufs=1)
            qkv_ps = pool(name="qkv_ps", bufs=1, space="PSUM")
            sc_ps = pool(name="sc_ps", bufs=2, space="PSUM")
            av_ps = pool(name="av_ps", bufs=1, space="PSUM")
            den_ps = pool(name="den_ps", bufs=1, space="PSUM")
            pj_ps = pool(name="pj_ps", bufs=1, space="PSUM")
            facc_ps = pool(name="facc_ps", bufs=2, space="PSUM")

            # ---- constants
            ones_st = consts.tile([1, 128], f32, tag="ones_st")
            nc.vector.memset(ones_st[:], 1.0)
            ones_row = consts.tile([1, 128], f32r, tag="ones_row")
            nc.scalar.copy(ones_row[:], ones_st[:])
            oe_st = consts.tile([1, 128], f32, tag="oe_st")
            nc.vector.memset(oe_st[:], 1.0 / NCORES)
            ones8_row = consts.tile([1, 128], f32r, tag="ones8_row")
            nc.scalar.copy(ones8_row[:], oe_st[:])
            onec_st = consts.tile([128, 1], f32, tag="onec_st")
            nc.vector.memset(onec_st[:], 1.0)
            ones_col = consts.tile([128, 1], bf16, tag="ones_col")
            nc.scalar.copy(ones_col[:], onec_st[:])
            eps_t = consts.tile([1, 1], f32, tag="eps_t")
            nc.vector.memset(eps_t[:], EPS)

            # ---- weights (persistent, bf16); wq up front, rest staged late
            wq_sb = []
            for g in range(KG):
                w = wpool.tile([128, 4, 3 * CL], bf16, name=f"wq{g}",
                               tag=f"wq{g}")
                nc.sync.dma_start(w[:], wqkv[g])
                wq_sb.append(w)

            wp_sb, wf1_sb, wf2_sb, wm_sb = [], [], [], []

            def load_wp():
                if wp_sb:
                    return
                for lc in range(HL):
                    w = wpool.tile([128, C], bf16, name=f"wp{lc}",
                                   tag=f"wp{lc}")
                    nc.sync.dma_start(w[:], wproj[lc * 128:(lc + 1) * 128, :])
                    wp_sb.append(w)

            def load_wfm():
                if wf1_sb:
                    return
                for g in range(KG):
                    w1 = wpool.tile([128, 4, DL], bf16, name=f"wf1_{g}",
                                    tag=f"wf1_{g}")
                    nc.sync.dma_start(w1[:], wfc1[g])
                    wf1_sb.append(w1)
                    w2 = wpool.tile([128, 4, DL], bf16, name=f"wf2_{g}",
                                    tag=f"wf2_{g}")
                    nc.sync.dma_start(w2[:], wfc2[g])
                    wf2_sb.append(w2)
                off = 0
                for oc, osz in enumerate(FCO):
                    w = wpool.tile([osz, C], bf16, name=f"wm{oc}",
                                   tag=f"wm{oc}")
                    nc.sync.dma_start(w[:], wmp[off:off + osz, :])
                    wm_sb.append(w)
                    off += osz

            # ---- persistent K / V
            kt = [pers.tile([128, T], bf16, name=f"k{h}", tag=f"k{h}")
                  for h in range(HL)]
            v_sb = [pers.tile([128, KC, 128], bf16, name=f"v{h}", tag=f"v{h}")
                    for h in range(HL)]

            for rep in range(reps):
                arin = [nc.dram_tensor(f"arin{i}_{rep}", [KC, 128, 512],
                                       bf16).ap() for i in range(NT4)]
                ar_kw = {"addr_space": "Shared"} if collectives else {}
                arout = [nc.dram_tensor(f"arout{i}_{rep}", [KC, 128, 512],
                                        bf16, **ar_kw).ap()
                         for i in range(NT4)]
                rsin = [nc.dram_tensor(f"rsin{i}_{rep}", [KC, 128, 512],
                                       bf16).ap() for i in range(NT4)]
                rsout = [nc.dram_tensor(f"rsout{i}_{rep}",
                                        [C // NCORES, 512], bf16).ap()
                         for i in range(NT4)]

                qtiles = {}
                mlp_state = {}

                def emit_qkv(t4):
                    xh4s = []
                    for g in range(KG):
                        xh = xh4p.tile([128, 4, 512], bf16, name="xh4",
                                       tag="xh4")
                        nc.sync.dma_start(xh[:], xh_d[g, t4])
                        xh4s.append(xh)
                    for w in range(6):
                        ps = qkv_ps.tile([128, 512], f32, name="qp",
                                         tag="qkv", bufs=1)
                        for kc in range(KC):
                            g, cc4 = kc // 4, kc % 4
                            nc.tensor.matmul(
                                ps[:],
                                wq_sb[g][:, cc4, w * 128:(w + 1) * 128],
                                xh4s[g][:, cc4, :], start=(kc == 0),
                                stop=(kc == KC - 1), skip_group_check=True)
                        sl = slice(t4 * 512, (t4 + 1) * 512)
                        if w < 2:      # q head w, per-chunk tile
                            qt = qtp.tile([128, 512], bf16, name=f"q{w}",
                                          tag=f"q{w}")
                            nc.scalar.copy(qt[:], ps[:])
                            qtiles[(t4, w)] = qt
                        elif w < 4:    # k head w-2
                            nc.scalar.copy(kt[w - 2][:, sl], ps[:])
                        else:          # v head w-4: transpose via XBAR DMA
                            h = w - 4
                            vt = vtp.tile([128, 512], bf16, name="vt",
                                          tag=f"vt{h}")
                            nc.scalar.copy(vt[:], ps[:])
                            nc.sync.dma_start_transpose(
                                v_sb[h][:, t4 * 4:(t4 + 1) * 4, :], vt[:])

                def emit_attn(t4, h):
                    ntk = (t4 + 1) * 4
                    qt = qtiles[(t4, h)]
                    algs = []
                    for jg in range(ntk // 4):
                        alg = al4p.tile([128, 4, 512], bf16, name="al4",
                                        tag="al4")
                        nc.sync.dma_start(alg[:], alibiT[h, jg, t4])
                        algs.append(alg)
                    po = av_ps.tile([128, 512], f32, name="po", tag="po",
                                    bufs=1)
                    den = den_ps.tile([1, 512], f32, name="den",
                                      tag="den", bufs=1)
                    prs = {}

                    def _scores(j):
                        tsl = slice(j * 128, (j + 1) * 128)
                        ps_s = sc_ps.tile([128, 512], f32, name="ps_s",
                                          tag="sc", bufs=2)
                        nc.tensor.matmul(ps_s[:], kt[h][:, tsl], qt[:],
                                         start=True, stop=True,
                                         skip_group_check=True)
                        es = ssbp.tile([128, 512], bf16, name="es",
                                       tag="s_sb")
                        nc.scalar.activation(es[:], ps_s[:], FA.Exp)
                        pr = prp.tile([128, 512], bf16, name="pr", tag="pr")
                        nc.vector.tensor_mul(pr[:], es[:],
                                             algs[j // 4][:, j % 4, :])
                        prs[j] = pr

                    def _av(j):
                        pr = prs.pop(j)
                        nc.tensor.matmul(den[:], ones_col[:], pr[:],
                                         start=(j == 0), stop=(j == ntk - 1),
                                         skip_group_check=True)
                        nc.tensor.matmul(po[:], v_sb[h][:, j, :], pr[:],
                                         start=(j == 0), stop=(j == ntk - 1),
                                         skip_group_check=True)

                    LOOK = 2
                    for j in range(ntk):
                        _scores(j)
                        if j >= LOOK:
                            _av(j - LOOK)
                    for j in range(max(0, ntk - LOOK), ntk):
                        _av(j)

                    rc = smalls.tile([1, 512], f32, name="rc", tag="rc")
                    nc.vector.reciprocal(rc[:], den[:])
                    rcr = smalls.tile([1, 512], f32r, name="rcr", tag="rcr",
                                      bufs=1)
                    nc.scalar.copy(rcr[:], rc[:])
                    pbc = sc_ps.tile([128, 512], f32, name="pbc",
                                     tag="sc", bufs=2)
                    nc.tensor.matmul(pbc[:], ones_row[:], rcr[:],
                                     start=True, stop=True,
                                     skip_group_check=True)
                    rb = rbp.tile([128, 512], bf16, name="rb", tag="rb")
                    nc.scalar.copy(rb[:], pbc[:])
                    yt = ytp.tile([128, 512], bf16, name="yt", tag="yt")
                    nc.vector.tensor_mul(yt[:], po[:], rb[:])
                    qtiles[(t4, h)] = None
                    qtiles[(t4, h, "y")] = yt

                def emit_proj(t4):
                    yts = [qtiles[(t4, h, "y")] for h in range(HL)]
                    for cp in range(KC // 2):
                        g, half = cp // 2, cp % 2
                        xtd = xtdp.tile([128, 2, 512], bf16, name="xtd",
                                        tag="xtd")
                        nc.sync.dma_start(
                            xtd[:], xt[g, t4][:, half * 2:half * 2 + 2, :])
                        ob = obp.tile([128, 2, 512], bf16, name="ob",
                                      tag="ob")
                        for c in range(2):
                            cc = cp * 2 + c
                            pj = pj_ps.tile([128, 512], f32, name="pj",
                                            tag="pj", bufs=1)
                            for lc in range(HL):
                                nc.tensor.matmul(
                                    pj[:],
                                    wp_sb[lc][:, cc * 128:(cc + 1) * 128],
                                    yts[lc][:], start=(lc == 0),
                                    stop=(lc == HL - 1))
                            nc.vector.scalar_tensor_tensor(
                                ob[:, c, :], xtd[:, c, :], 1.0 / NCORES,
                                pj[:], op0=ALU.mult, op1=ALU.add)
                        nc.sync.dma_start(
                            arin[t4][cp * 2:cp * 2 + 2].transpose([1, 0, 2]),
                            ob[:])
                    if collectives:
                        nc.gpsimd.collective_compute(
                            "AllReduce", ALU.add, ins=[arin[t4][:]],
                            outs=[arout[t4][:]], replica_groups=RG)
                    else:
                        nc.sync.dma_start(arout[t4][:], arin[t4][:])

                def emit_mlp_stats(t4):
                    psst = qkv_ps.tile([128, 512], f32, name="pss",
                                       tag="qkv", bufs=1)
                    pss = psst[0:1, :]
                    for gp in range(KC // 2):
                        x2t = x2sp.tile([128, 2, 512], bf16, name="x2t",
                                        tag="x2s")
                        nc.sync.dma_start(
                            x2t[:],
                            arout[t4][gp * 2:gp * 2 + 2].transpose([1, 0, 2]))
                        for c in range(2):
                            cc = gp * 2 + c
                            xq = xqp.tile([128, 512], bf16, name="xq",
                                          tag="xq")
                            nc.scalar.activation(xq[:], x2t[:, c, :],
                                                 FA.Square)
                            nc.tensor.matmul(pss[:], ones_col[:], xq[:],
                                             start=(cc == 0),
                                             stop=(cc == KC - 1),
                                             skip_group_check=True)
                    sq = smalls.tile([1, 512], f32, name="sq", tag="sq")
                    nc.scalar.activation(sq[:], pss[:], FA.Sqrt,
                                         bias=eps_t[:], scale=1.0 / C)
                    rc2 = smalls.tile([1, 512], f32, name="rc2", tag="rc2")
                    nc.vector.reciprocal(rc2[:], sq[:])
                    mr1 = smalls.tile([1, 512], f32r, name="mr1", tag="mr",
                                      bufs=1)
                    nc.scalar.copy(mr1[:], rc2[:])
                    pb2 = facc_ps.tile([128, 512], f32, name="pb2",
                                       tag="facc", bufs=2)
                    nc.tensor.matmul(pb2[:], ones_row[:], mr1[:],
                                     start=True, stop=True,
                                     skip_group_check=True)
                    rb2 = rbp.tile([128, 512], bf16, name="rb2", tag="rb2",
                                   bufs=1)
                    nc.scalar.copy(rb2[:], pb2[:])
                    # sqb2 = rms/8 broadcast, to reconstruct x2/8 at pm time
                    mr2 = smalls.tile([1, 512], f32r, name="mr2", tag="mr",
                                      bufs=1)
                    nc.scalar.copy(mr2[:], sq[:])
                    pb3 = facc_ps.tile([128, 512], f32, name="pb3",
                                       tag="facc", bufs=2)
                    nc.tensor.matmul(pb3[:], ones8_row[:], mr2[:],
                                     start=True, stop=True,
                                     skip_group_check=True)
                    sqb2 = rbp.tile([128, 512], bf16, name="sqb2",
                                    tag="sqb2", bufs=1)
                    nc.scalar.copy(sqb2[:], pb3[:])

                    xh2 = []
                    for gp in range(KC // 2):
                        x2t = x2bp.tile([128, 2, 512], bf16, name="x2b",
                                        tag="x2b")
                        nc.sync.dma_start(
                            x2t[:],
                            arout[t4][gp * 2:gp * 2 + 2].transpose([1, 0, 2]))
                        for c in range(2):
                            xh = xh2p.tile([128, 512], bf16, name="xh2",
                                           tag="xh2")
                            nc.vector.tensor_mul(xh[:], x2t[:, c, :], rb2[:])
                            xh2.append(xh)
                    mlp_state[t4] = {"xh2": xh2, "g": [], "sqb2": sqb2}

                def emit_mlp_fc(t4, ocs):
                    xh2 = mlp_state[t4]["xh2"]
                    g_tiles = mlp_state[t4]["g"]
                    off = sum(FCO[:ocs[0]])
                    for oc in ocs:
                        osz = FCO[oc]
                        pa = facc_ps.tile([128, 512], f32, name="pa",
                                          tag="facc", bufs=2)
                        for kc in range(KC):
                            g, cc4 = kc // 4, kc % 4
                            nc.tensor.matmul(
                                pa[:osz],
                                wf1_sb[g][:, cc4, off:off + osz],
                                xh2[kc][:], start=(kc == 0),
                                stop=(kc == KC - 1))
                        a_sb = asbp.tile([128, 512], bf16, name="a_sb",
                                         tag="a_sb")
                        nc.scalar.activation(a_sb[:osz], pa[:osz], FA.Silu)
                        pb_ = facc_ps.tile([128, 512], f32, name="pb_",
                                           tag="facc", bufs=2)
                        for kc in range(KC):
                            g, cc4 = kc // 4, kc % 4
                            nc.tensor.matmul(
                                pb_[:osz],
                                wf2_sb[g][:, cc4, off:off + osz],
                                xh2[kc][:], start=(kc == 0),
                                stop=(kc == KC - 1))
                        g_sb = gpool.tile([128, 512], bf16, name="g_sb",
                                          tag="g_sb")
                        nc.vector.tensor_mul(g_sb[:osz], pb_[:osz],
                                             a_sb[:osz])
                        g_tiles.append(g_sb)
                        off += osz

                def emit_mlp_pm(t4):
                    g_tiles = mlp_state[t4]["g"]
                    xh2 = mlp_state[t4]["xh2"]
                    sqb2 = mlp_state[t4]["sqb2"]
                    for cp in range(KC // 2):
                        ob = obp.tile([128, 2, 512], bf16, name="ob2",
                                      tag="ob")
                        for c in range(2):
                            cc = cp * 2 + c
                            pm = facc_ps.tile([128, 512], f32, name="pm",
                                              tag="facc", bufs=2)
                            for oc, osz in enumerate(FCO):
                                nc.tensor.matmul(
                                    pm[:],
                                    wm_sb[oc][:, cc * 128:(cc + 1) * 128],
                                    g_tiles[oc][:osz], start=(oc == 0),
                                    stop=(oc == len(FCO) - 1))
                            tmp = tmpp.tile([128, 512], bf16, name="tmp",
                                            tag="tmp")
                            nc.vector.tensor_mul(tmp[:], xh2[cc][:],
                                                 sqb2[:])
                            nc.vector.tensor_tensor(ob[:, c, :], tmp[:],
                                                    pm[:], op=ALU.add)
                        nc.sync.dma_start(
                            rsin[t4][cp * 2:cp * 2 + 2].transpose([1, 0, 2]),
                            ob[:])
                    if collectives:
                        nc.gpsimd.collective_compute(
                            "ReduceScatter", ALU.add, ins=[rsin[t4][:]],
                            outs=[rsout[t4][:]], replica_groups=RG)
                    else:
                        nc.sync.dma_start(
                            rsout[t4][:],
                            rsin[t4][:C // NCORES // 128])
                    nc.sync.dma_start(out[:, t4 * 512:(t4 + 1) * 512],
                                      rsout[t4][:])

                for t in range(NT4 + 1):
                    if t < NT4:
                        emit_qkv(t)
                        if t == 0:
                            load_wp()
                    if t > 0:
                        emit_mlp_stats(t - 1)
                    if t < NT4:
                        emit_attn(t, 0)
                    if t > 0:
                        emit_mlp_fc(t - 1, [0, 1, 2])
                    if t < NT4:
                        emit_attn(t, 1)
                    if t > 0:
                        emit_mlp_fc(t - 1, [3, 4, 5])
                        emit_mlp_pm(t - 1)
                    if t < NT4:
                        emit_proj(t)
                        if t == 0:
                            load_wfm()

    nc.compile()
    return nc


_NC = None


def _get_nc():
    global _NC
    if _NC is None:
        _NC = build()
    return _NC


def _tile4(a2d):
    """[R, T] -> [R//512, NT4, 128, 4, 512]: (G, t4, p, c, q)."""
    R = a2d.shape[0]
    return np.ascontiguousarray(
        a2d.reshape(R // 512, 4, 128, NT4, 512).transpose(0, 3, 2, 1, 4))


def _wtile(w2d):
    """[C, W] -> [KG, 128, 4, W]."""
    W = w2d.shape[1]
    return np.ascontiguousarray(
        w2d.reshape(KG, 4, 128, W).transpose(0, 2, 1, 3))


def make_in_maps(x, alibi, w_attn, w_proj, w_fc1, w_fc2, w_mlp_proj,
                 rms1_scale, rms2_scale):
    x = np.asarray(x, dtype=np.float32)
    alibi = np.asarray(alibi, dtype=np.float32)
    w_attn = np.asarray(w_attn, dtype=np.float32)
    w_proj = np.asarray(w_proj, dtype=np.float32)
    w_fc1 = np.asarray(w_fc1, dtype=np.float32)
    w_fc2 = np.asarray(w_fc2, dtype=np.float32)
    w_mlp_proj = np.asarray(w_mlp_proj, dtype=np.float32)
    rms1_scale = np.asarray(rms1_scale, dtype=np.float32)
    rms2_scale = np.asarray(rms2_scale, dtype=np.float32)

    xT = x[0].T                                            # [C, T]
    ms = np.mean(x[0].astype(np.float32) ** 2, axis=-1)    # [T]
    rs1 = (1.0 / np.sqrt(ms + EPS)).astype(np.float32)
    xt = _tile4(xT).astype(ml_dtypes.bfloat16)
    xh_d = _tile4(xT * rs1[None, :]).astype(ml_dtypes.bfloat16)

    # fold rms scales into the weight rows; 1/sqrt(hd) into q AND k columns
    wA = w_attn * rms1_scale[:, None]
    sq = np.float32(ISQ ** 0.5)
    wf1 = w_fc1 * rms2_scale[:, None]
    wf2 = w_fc2 * rms2_scale[:, None]

    # causal mask folded into the (transposed, exponentiated) alibi
    tk = np.arange(T)[:, None]
    q = np.arange(T)[None, :]
    cmaskT = np.where(tk <= q, 0.0, NEG).astype(np.float32)

    in_maps = []
    for r in range(NCORES):
        wqkv_r = np.concatenate(
            [wA[:, r * CL:(r + 1) * CL] * sq,
             wA[:, C + r * CL:C + (r + 1) * CL] * sq,
             wA[:, 2 * C + r * CL:2 * C + (r + 1) * CL]], axis=1)
        alE = np.exp(alibi[r * HL:(r + 1) * HL].transpose(0, 2, 1)
                     + cmaskT[None])
        # [HL, Tk, Tq] -> [HL, Tk//512, NT4, 128, 4, 512]
        alE = np.ascontiguousarray(
            alE.reshape(HL, T // 512, 4, 128, NT4, 512)
               .transpose(0, 1, 4, 3, 2, 5)).astype(ml_dtypes.bfloat16)
        in_maps.append({
            "xt": xt,
            "xh_d": xh_d,
            "wqkv": _wtile(wqkv_r).astype(ml_dtypes.bfloat16),
            "alibiT": alE,
            "wproj": np.ascontiguousarray(
                w_proj[r * CL:(r + 1) * CL, :]).astype(ml_dtypes.bfloat16),
            "wfc1": _wtile(wf1[:, r * DL:(r + 1) * DL]
                           ).astype(ml_dtypes.bfloat16),
            "wfc2": _wtile(wf2[:, r * DL:(r + 1) * DL]
                           ).astype(ml_dtypes.bfloat16),
            "wmp": np.ascontiguousarray(
                w_mlp_proj[r * DL:(r + 1) * DL, :]).astype(ml_dtypes.bfloat16),
        })
    return in_maps


def assemble(results):
    full = np.empty((T, C), dtype=np.float32)
    for r in range(NCORES):
        full[:, r * (C // NCORES):(r + 1) * (C // NCORES)] = \
            results[r]["out"].astype(np.float32).T
    return full[None, :, :]


def kernel(x, alibi, w_attn, w_proj, w_fc1, w_fc2, w_mlp_proj,
           rms1_scale, rms2_scale):
    nc = _get_nc()
    in_maps = make_in_maps(x, alibi, w_attn, w_proj, w_fc1, w_fc2, w_mlp_proj,
                           rms1_scale, rms2_scale)
    res = run_bass_kernel_spmd(nc, in_maps, core_ids=list(range(NCORES)))
    return assemble(res.results)


# revision 31
# speedup vs baseline: 1.1018x; 1.1018x over previous
"""Tensor-parallel transformer block (attention + MLP with RMSNorm) on 8 TRN2 NeuronCores.

Single-scope software-pipelined design (v3, batched DMA):
  - TP-8: c_attn column-sharded by heads (2 heads/core), c_proj row-sharded,
    fc1/fc2 column-sharded (704/core), mlp proj row-sharded. All bf16.
  - One TileContext scope; per-chunk emission interleaves attn(t) with
    MLP(t-1) so the tile scheduler hides the AllReduce and keeps PE busy.
  - DMA instruction count minimized: host pre-tiles x-hat / x / exp(alibi) /
    weights into [*, 128, batch, 512] layouts so each DMA moves 2-4 tiles
    contiguously per partition (HWDGE queue service is ~0.6us/op and was
    pacing the pipeline).
  - rmsnorm1 folded into the host-supplied x-hat; 1/sqrt(hd) folded into the
    q/k weight columns; causal mask+alibi pre-exponentiated host-side so
    probs = exp(scores) * alibi_exp (DVE 2x bf16 multiply, no add chain).
  - MLP residual reconstructs x2 = xh2 * rms (sqb2 broadcast) to skip a
    third arout read pass.
  - PSUM (8 banks): qkv(1, also rms2-stats row) + scores(2, also recip
    broadcasts) + po(1) + den(1) + pj(1) + facc(2: fc1/fc2/mlp-proj/pb2).
  - AllReduce carries attn_partial + x/8 (bf16); ReduceScatter carries
    mlp_partial + x2/8 (bf16), C-sharded; host transposes/concats shards.
"""
import numpy as np
import ml_dtypes

import concourse.bass as bass
import concourse.tile as tile
from concourse import bacc, mybir
from concourse.bass_utils import run_bass_kernel_spmd

f32 = mybir.dt.float32
f32r = mybir.dt.float32r
bf16 = mybir.dt.bfloat16
FA = mybir.ActivationFunctionType
ALU = mybir.AluOpType

NCORES = 8
T = 2048
C = 2048
NH = 16
HD = 128
HL = NH // NCORES          # 2 local heads
CL = HL * HD               # 256 local attn dims
DFF = 5632
DL = DFF // NCORES         # 704 local ffn dims
EPS = 1e-5
ISQ = float(1.0 / np.sqrt(HD))
NEG = -30000.0
KC = C // 128              # 16 contraction chunks
KG = KC // 4               # 4 groups of 4 chunks
NT4 = T // 512             # 4 T-chunks of 512
FCO = [128] * 5 + [64]     # DL partition chunks (704 = 5*128 + 64)
RG = [list(range(NCORES))]


def build(reps=1, collectives=True):
    nc = bacc.Bacc("TRN2", target_bir_lowering=False, debug=False,
                   enable_asserts=False,
                   num_devices=NCORES if collectives else 1)

    xt = nc.dram_tensor("xt", [KG, NT4, 128, 4, 512], bf16,
                        kind="ExternalInput").ap()
    xh_d = nc.dram_tensor("xh_d", [KG, NT4, 128, 4, 512], bf16,
                          kind="ExternalInput").ap()
    wqkv = nc.dram_tensor("wqkv", [KG, 128, 4, 3 * CL], bf16,
                          kind="ExternalInput").ap()
    alibiT = nc.dram_tensor("alibiT", [HL, T // 512, NT4, 128, 4, 512],
                            bf16, kind="ExternalInput").ap()
    wproj = nc.dram_tensor("wproj", [CL, C], bf16, kind="ExternalInput").ap()
    wfc1 = nc.dram_tensor("wfc1", [KG, 128, 4, DL], bf16,
                          kind="ExternalInput").ap()
    wfc2 = nc.dram_tensor("wfc2", [KG, 128, 4, DL], bf16,
                          kind="ExternalInput").ap()
    wmp = nc.dram_tensor("wmp", [DL, C], bf16, kind="ExternalInput").ap()
    out = nc.dram_tensor("out", [NT4, C // NCORES, 512], bf16,
                         kind="ExternalOutput").ap()

    from contextlib import ExitStack
    with tile.TileContext(nc) as tc:
        with ExitStack() as stack:
            pool = lambda *a, **k: stack.enter_context(tc.tile_pool(*a, **k))
            consts = pool(name="consts", bufs=1)
            wpool = pool(name="weights", bufs=1)
            pers = pool(name="persist", bufs=1)
            qtp = pool(name="qt", bufs=2)
            xh4p = pool(name="xh4", bufs=4)
            al4p = pool(name="al4", bufs=2)
            vtp = pool(name="vt", bufs=1)
            ssbp = pool(name="ssb", bufs=1)
            prp = pool(name="pr", bufs=3)
            ytp = pool(name="yt", bufs=2)
            xtdp = pool(name="xtd2", bufs=1)
            obp = pool(name="ob2g", bufs=2)
            x2sp = pool(name="x2s2", bufs=2)
            x2bp = pool(name="x2b2", bufs=2)
            xqp = pool(name="xq", bufs=1)
            xh2p = pool(name="xh2", bufs=16)
            asbp = pool(name="asb", bufs=1)
            gpool = pool(name="g", bufs=6)
            rbp = pool(name="rb", bufs=1)
            tmpp = pool(name="tmp", bufs=1)
            smalls = pool(name="smalls", bufs=1)
            qkv_ps = pool(name="qkv_ps", bufs=1, space="PSUM")
            sc_ps = pool(name="sc_ps", bufs=2, space="PSUM")
            av_ps = pool(name="av_ps", bufs=1, space="PSUM")
            den_ps = pool(name="den_ps", bufs=1, space="PSUM")
            pj_ps = pool(name="pj_ps", bufs=1, space="PSUM")
            facc_ps = pool(name="facc_ps", bufs=2, space="PSUM")

            # ---- constants
            ones_st = consts.tile([1, 128], f32, tag="ones_st")
            nc.vector.memset(ones_st[:], 1.0)
            ones_row = consts.tile([1, 128], f32r, tag="ones_row")
            nc.scalar.copy(ones_row[:], ones_st[:])
            oe_st = consts.tile([1, 128], f32, tag="oe_st")
            nc.vector.memset(oe_st[:], 1.0 / NCORES)
            ones8_row = consts.tile([1, 128], f32r, tag="ones8_row")
            nc.scalar.copy(ones8_row[:], oe_st[:])
            onec_st = consts.tile([128, 1], f32, tag="onec_st")
            nc.vector.memset(onec_st[:], 1.0)
            ones_col = consts.tile([128, 1], bf16, tag="ones_col")
            nc.scalar.copy(ones_col[:], onec_st[:])
            eps_t = consts.tile([1, 1], f32, tag="eps_t")
            nc.vector.memset(eps_t[:], EPS)

            # ---- weights (persistent, bf16); wq up front, rest staged late
            wq_sb = []
            for g in range(KG):
                w = wpool.tile([128, 4, 3 * CL], bf16, name=f"wq{g}",
                               tag=f"wq{g}")
                nc.sync.dma_start(w[:], wqkv[g])
                wq_sb.append(w)

            wp_sb, wf1_sb, wf2_sb, wm_sb = [], [], [], []

            def load_wp():
                if wp_sb:
                    return
                for lc in range(HL):
                    w = wpool.tile([128, C], bf16, name=f"wp{lc}",
                                   tag=f"wp{lc}")
                    nc.sync.dma_start(w[:], wproj[lc * 128:(lc + 1) * 128, :])
                    wp_sb.append(w)

            def load_wfm():
                if wf1_sb:
                    return
                for g in range(KG):
                    w1 = wpool.tile([128, 4, DL], bf16, name=f"wf1_{g}",
                                    tag=f"wf1_{g}")
                    nc.sync.dma_start(w1[:], wfc1[g])
                    wf1_sb.append(w1)
                    w2 = wpool.tile([128, 4, DL], bf16, name=f"wf2_{g}",
                                    tag=f"wf2_{g}")
                    nc.sync.dma_start(w2[:], wfc2[g])
                    wf2_sb.append(w2)
                off = 0
                for oc, osz in enumerate(FCO):
                    w = wpool.tile([osz, C], bf16, name=f"wm{oc}",
                                   tag=f"wm{oc}")
                    nc.sync.dma_start(w[:], wmp[off:off + osz, :])
                    wm_sb.append(w)
                    off += osz

            # ---- persistent K / V
            kt = [pers.tile([128, T], bf16, name=f"k{h}", tag=f"k{h}")
                  for h in range(HL)]
            v_sb = [pers.tile([128, KC, 128], bf16, name=f"v{h}", tag=f"v{h}")
                    for h in range(HL)]

            for rep in range(reps):
                arin = [nc.dram_tensor(f"arin{i}_{rep}", [KC, 128, 512],
                                       bf16).ap() for i in range(NT4)]
                ar_kw = {"addr_space": "Shared"} if collectives else {}
                arout = [nc.dram_tensor(f"arout{i}_{rep}", [KC, 128, 512],
                                        bf16, **ar_kw).ap()
                         for i in range(NT4)]
                rsin = [nc.dram_tensor(f"rsin{i}_{rep}", [KC, 128, 512],
                                       bf16).ap() for i in range(NT4)]
                rsout = [nc.dram_tensor(f"rsout{i}_{rep}",
                                        [C // NCORES, 512], bf16).ap()
                         for i in range(NT4)]

                qtiles = {}
                mlp_state = {}

                def emit_qkv(t4):
                    xh4s = []
                    for g in range(KG):
                        xh = xh4p.tile([128, 4, 512], bf16, name="xh4",
                                       tag="xh4")
                        nc.sync.dma_start(xh[:], xh_d[g, t4])
                        xh4s.append(xh)
                    for w in range(6):
                        ps = qkv_ps.tile([128, 512], f32, name="qp",
                                         tag="qkv", bufs=1)
                        for kc in range(KC):
                            g, cc4 = kc // 4, kc % 4
                            nc.tensor.matmul(
                                ps[:],
                                wq_sb[g][:, cc4, w * 128:(w + 1) * 128],
                                xh4s[g][:, cc4, :], start=(kc == 0),
                                stop=(kc == KC - 1), skip_group_check=True)
                        sl = slice(t4 * 512, (t4 + 1) * 512)
                        if w < 2:      # q head w, per-chunk tile
                            qt = qtp.tile([128, 512], bf16, name=f"q{w}",
                                          tag=f"q{w}")
                            nc.scalar.copy(qt[:], ps[:])
                            qtiles[(t4, w)] = qt
                        elif w < 4:    # k head w-2
                            nc.scalar.copy(kt[w - 2][:, sl], ps[:])
                        else:          # v head w-4: transpose via XBAR DMA
                            h = w - 4
                            vt = vtp.tile([128, 512], bf16, name="vt",
                                          tag=f"vt{h}")
                            nc.scalar.copy(vt[:], ps[:])
                            nc.sync.dma_start_transpose(
                                v_sb[h][:, t4 * 4:(t4 + 1) * 4, :], vt[:])

                def emit_attn(t4, h):
                    ntk = (t4 + 1) * 4
                    qt = qtiles[(t4, h)]
                    algs = []
                    for jg in range(ntk // 4):
                        alg = al4p.tile([128, 4, 512], bf16, name="al4",
                                        tag="al4")
                        nc.sync.dma_start(alg[:], alibiT[h, jg, t4])
                        algs.append(alg)
                    po = av_ps.tile([128, 512], f32, name="po", tag="po",
                                    bufs=1)
                    den = den_ps.tile([1, 512], f32, name="den",
                                      tag="den", bufs=1)
                    prs = {}

                    def _scores(j):
                        tsl = slice(j * 128, (j + 1) * 128)
                        ps_s = sc_ps.tile([128, 512], f32, name="ps_s",
                                          tag="sc", bufs=2)
                        nc.tensor.matmul(ps_s[:], kt[h][:, tsl], qt[:],
                                         start=True, stop=True,
                                         skip_group_check=True)
                        es = ssbp.tile([128, 512], bf16, name="es",
                                       tag="s_sb")
                        nc.scalar.activation(es[:], ps_s[:], FA.Exp)
                        pr = prp.tile([128, 512], bf16, name="pr", tag="pr")
                        nc.vector.tensor_mul(pr[:], es[:],
                                             algs[j // 4][:, j % 4, :])
                        prs[j] = pr

                    def _av(j):
                        pr = prs.pop(j)
                        nc.tensor.matmul(den[:], ones_col[:], pr[:],
                                         start=(j == 0), stop=(j == ntk - 1),
                                         skip_group_check=True)
                        nc.tensor.matmul(po[:], v_sb[h][:, j, :], pr[:],
                                         start=(j == 0), stop=(j == ntk - 1),
                                         skip_group_check=True)

                    LOOK = 2
                    for j in range(ntk):
                        _scores(j)
                        if j >= LOOK:
                            _av(j - LOOK)
                    for j in range(max(0, ntk - LOOK), ntk):
                        _av(j)

                    rc = smalls.tile([1, 512], f32, name="rc", tag="rc")
                    nc.vector.reciprocal(rc[:], den[:])
                    rcr = smalls.tile([1, 512], f32r, name="rcr", tag="rcr",
                                      bufs=1)
                    nc.scalar.copy(rcr[:], rc[:])
                    pbc = sc_ps.tile([128, 512], f32, name="pbc",
                                     tag="sc", bufs=2)
                    nc.tensor.matmul(pbc[:], ones_row[:], rcr[:],
                                     start=True, stop=True,
                                     skip_group_check=True)
                    rb = rbp.tile([128, 512], bf16, name="rb", tag="rb")
                    nc.scalar.copy(rb[:], pbc[:])
                    yt = ytp.tile([128, 512], bf16, name="yt", tag="yt")
                    nc.vector.tensor_mul(yt[:], po[:], rb[:])
                    qtiles[(t4, h)] = None
                    qtiles[(t4, h, "y")] = yt

                def emit_proj(t4):
                    yts = [qtiles[(t4, h, "y")] for h in range(HL)]
                    for cp in range(KC // 2):
                        g, half = cp // 2, cp % 2
                        xtd = xtdp.tile([128, 2, 512], bf16, name="xtd",
                                        tag="xtd")
                        nc.sync.dma_start(
                            xtd[:], xt[g, t4][:, half * 2:half * 2 + 2, :])
                        ob = obp.tile([128, 2, 512], bf16, name="ob",
                                      tag="ob")
                        for c in range(2):
                            cc = cp * 2 + c
                            pj = pj_ps.tile([128, 512], f32, name="pj",
                                            tag="pj", bufs=1)
                            for lc in range(HL):
                                nc.tensor.matmul(
                                    pj[:],
                                    wp_sb[lc][:, cc * 128:(cc + 1) * 128],
                                    yts[lc][:], start=(lc == 0),
                                    stop=(lc == HL - 1))
                            nc.vector.scalar_tensor_tensor(
                                ob[:, c, :], xtd[:, c, :], 1.0 / NCORES,
                                pj[:], op0=ALU.mult, op1=ALU.add)
                            nc.sync.dma_start(arin[t4][cp * 2 + c],
                                              ob[:, c, :])
                    if collectives:
                        nc.gpsimd.collective_compute(
                            "AllReduce", ALU.add, ins=[arin[t4][:]],
                            outs=[arout[t4][:]], replica_groups=RG)
                    else:
                        nc.sync.dma_start(arout[t4][:], arin[t4][:])

                def emit_mlp_stats(t4):
                    psst = qkv_ps.tile([128, 512], f32, name="pss",
                                       tag="qkv", bufs=1)
                    pss = psst[0:1, :]
                    for cc in range(KC):
                        x2t = x2sp.tile([128, 512], bf16, name="x2t",
                                        tag="x2s", bufs=4)
                        nc.sync.dma_start(x2t[:], arout[t4][cc])
                        if True:
                            xq = xqp.tile([128, 512], bf16, name="xq",
                                          tag="xq")
                            nc.scalar.activation(xq[:], x2t[:],
                                                 FA.Square)
                            nc.tensor.matmul(pss[:], ones_col[:], xq[:],
                                             start=(cc == 0),
                                             stop=(cc == KC - 1),
                                             skip_group_check=True)
                    sq = smalls.tile([1, 512], f32, name="sq", tag="sq")
                    nc.scalar.activation(sq[:], pss[:], FA.Sqrt,
                                         bias=eps_t[:], scale=1.0 / C)
                    rc2 = smalls.tile([1, 512], f32, name="rc2", tag="rc2")
                    nc.vector.reciprocal(rc2[:], sq[:])
                    mr1 = smalls.tile([1, 512], f32r, name="mr1", tag="mr",
                                      bufs=1)
                    nc.scalar.copy(mr1[:], rc2[:])
                    pb2 = facc_ps.tile([128, 512], f32, name="pb2",
                                       tag="facc", bufs=2)
                    nc.tensor.matmul(pb2[:], ones_row[:], mr1[:],
                                     start=True, stop=True,
                                     skip_group_check=True)
                    rb2 = rbp.tile([128, 512], bf16, name="rb2", tag="rb2",
                                   bufs=1)
                    nc.scalar.copy(rb2[:], pb2[:])
                    # sqb2 = rms/8 broadcast, to reconstruct x2/8 at pm time
                    mr2 = smalls.tile([1, 512], f32r, name="mr2", tag="mr",
                                      bufs=1)
                    nc.scalar.copy(mr2[:], sq[:])
                    pb3 = facc_ps.tile([128, 512], f32, name="pb3",
                                       tag="facc", bufs=2)
                    nc.tensor.matmul(pb3[:], ones8_row[:], mr2[:],
                                     start=True, stop=True,
                                     skip_group_check=True)
                    sqb2 = rbp.tile([128, 512], bf16, name="sqb2",
                                    tag="sqb2", bufs=1)
                    nc.scalar.copy(sqb2[:], pb3[:])

                    xh2 = []
                    for cc in range(KC):
                        x2t = x2bp.tile([128, 512], bf16, name="x2b",
                                        tag="x2b", bufs=4)
                        nc.sync.dma_start(x2t[:], arout[t4][cc])
                        xh = xh2p.tile([128, 512], bf16, name="xh2",
                                       tag="xh2")
                        nc.vector.tensor_mul(xh[:], x2t[:], rb2[:])
                        xh2.append(xh)
                    mlp_state[t4] = {"xh2": xh2, "g": [], "sqb2": sqb2}

                def emit_mlp_fc(t4, ocs):
                    xh2 = mlp_state[t4]["xh2"]
                    g_tiles = mlp_state[t4]["g"]
                    off = sum(FCO[:ocs[0]])
                    for oc in ocs:
                        osz = FCO[oc]
                        pa = facc_ps.tile([128, 512], f32, name="pa",
                                          tag="facc", bufs=2)
                        for kc in range(KC):
                            g, cc4 = kc // 4, kc % 4
                            nc.tensor.matmul(
                                pa[:osz],
                                wf1_sb[g][:, cc4, off:off + osz],
                                xh2[kc][:], start=(kc == 0),
                                stop=(kc == KC - 1))
                        a_sb = asbp.tile([128, 512], bf16, name="a_sb",
                                         tag="a_sb")
                        nc.scalar.activation(a_sb[:osz], pa[:osz], FA.Silu)
                        pb_ = facc_ps.tile([128, 512], f32, name="pb_",
                                           tag="facc", bufs=2)
                        for kc in range(KC):
                            g, cc4 = kc // 4, kc % 4
                            nc.tensor.matmul(
                                pb_[:osz],
                                wf2_sb[g][:, cc4, off:off + osz],
                                xh2[kc][:], start=(kc == 0),
                                stop=(kc == KC - 1))
                        g_sb = gpool.tile([128, 512], bf16, name="g_sb",
                                          tag="g_sb")
                        nc.vector.tensor_mul(g_sb[:osz], pb_[:osz],
                                             a_sb[:osz])
                        g_tiles.append(g_sb)
                        off += osz

                def emit_mlp_pm(t4):
                    g_tiles = mlp_state[t4]["g"]
                    xh2 = mlp_state[t4]["xh2"]
                    sqb2 = mlp_state[t4]["sqb2"]
                    for cp in range(KC // 2):
                        ob = obp.tile([128, 2, 512], bf16, name="ob2",
                                      tag="ob")
                        for c in range(2):
                            cc = cp * 2 + c
                            pm = facc_ps.tile([128, 512], f32, name="pm",
                                              tag="facc", bufs=2)
                            for oc, osz in enumerate(FCO):
                                nc.tensor.matmul(
                                    pm[:],
                                    wm_sb[oc][:, cc * 128:(cc + 1) * 128],
                                    g_tiles[oc][:osz], start=(oc == 0),
                                    stop=(oc == len(FCO) - 1))
                            tmp = tmpp.tile([128, 512], bf16, name="tmp",
                                            tag="tmp")
                            nc.vector.tensor_mul(tmp[:], xh2[cc][:],
                                                 sqb2[:])
                            nc.vector.tensor_tensor(ob[:, c, :], tmp[:],
                                                    pm[:], op=ALU.add)
                            nc.sync.dma_start(rsin[t4][cp * 2 + c],
                                              ob[:, c, :])
                    if collectives:
                        nc.gpsimd.collective_compute(
                            "ReduceScatter", ALU.add, ins=[rsin[t4][:]],
                            outs=[rsout[t4][:]], replica_groups=RG)
                    else:
                        nc.sync.dma_start(
                            rsout[t4][:],
                            rsin[t4][:C // NCORES // 128])
                    nc.sync.dma_start(out[t4], rsout[t4][:])

                for t in range(NT4 + 1):
                    if t < NT4:
                        emit_qkv(t)
                        if t == 0:
                            load_wp()
                    if t > 0:
                        emit_mlp_stats(t - 1)
                    if t < NT4:
                        emit_attn(t, 0)
                    if t > 0:
                        emit_mlp_fc(t - 1, [0, 1, 2])
                    if t < NT4:
                        emit_attn(t, 1)
                    if t > 0:
                        emit_mlp_fc(t - 1, [3, 4, 5])
                        emit_mlp_pm(t - 1)
                    if t < NT4:
                        emit_proj(t)
                        if t == 0:
                            load_wfm()

    nc.compile()
    return nc


_NC = None


def _get_nc():
    global _NC
    if _NC is None:
        _NC = build()
    return _NC


def _tile4(a2d):
    """[R, T] -> [R//512, NT4, 128, 4, 512]: (G, t4, p, c, q)."""
    R = a2d.shape[0]
    return np.ascontiguousarray(
        a2d.reshape(R // 512, 4, 128, NT4, 512).transpose(0, 3, 2, 1, 4))


def _wtile(w2d):
    """[C, W] -> [KG, 128, 4, W]."""
    W = w2d.shape[1]
    return np.ascontiguousarray(
        w2d.reshape(KG, 4, 128, W).transpose(0, 2, 1, 3))


def make_in_maps(x, alibi, w_attn, w_proj, w_fc1, w_fc2, w_mlp_proj,
                 rms1_scale, rms2_scale):
    x = np.asarray(x, dtype=np.float32)
    alibi = np.asarray(alibi, dtype=np.float32)
    w_attn = np.asarray(w_attn, dtype=np.float32)
    w_proj = np.asarray(w_proj, dtype=np.float32)
    w_fc1 = np.asarray(w_fc1, dtype=np.float32)
    w_fc2 = np.asarray(w_fc2, dtype=np.float32)
    w_mlp_proj = np.asarray(w_mlp_proj, dtype=np.float32)
    rms1_scale = np.asarray(rms1_scale, dtype=np.float32)
    rms2_scale = np.asarray(rms2_scale, dtype=np.float32)

    xT = x[0].T                                            # [C, T]
    ms = np.mean(x[0].astype(np.float32) ** 2, axis=-1)    # [T]
    rs1 = (1.0 / np.sqrt(ms + EPS)).astype(np.float32)
    xt = _tile4(xT).astype(ml_dtypes.bfloat16)
    xh_d = _tile4(xT * rs1[None, :]).astype(ml_dtypes.bfloat16)

    # fold rms scales into the weight rows; 1/sqrt(hd) into q AND k columns
    wA = w_attn * rms1_scale[:, None]
    sq = np.float32(ISQ ** 0.5)
    wf1 = w_fc1 * rms2_scale[:, None]
    wf2 = w_fc2 * rms2_scale[:, None]

    # causal mask folded into the (transposed, exponentiated) alibi
    tk = np.arange(T)[:, None]
    q = np.arange(T)[None, :]
    cmaskT = np.where(tk <= q, 0.0, NEG).astype(np.float32)

    in_maps = []
    for r in range(NCORES):
        wqkv_r = np.concatenate(
            [wA[:, r * CL:(r + 1) * CL] * sq,
             wA[:, C + r * CL:C + (r + 1) * CL] * sq,
             wA[:, 2 * C + r * CL:2 * C + (r + 1) * CL]], axis=1)
        alE = np.exp(alibi[r * HL:(r + 1) * HL].transpose(0, 2, 1)
                     + cmaskT[None])
        # [HL, Tk, Tq] -> [HL, Tk//512, NT4, 128, 4, 512]
        alE = np.ascontiguousarray(
            alE.reshape(HL, T // 512, 4, 128, NT4, 512)
               .transpose(0, 1, 4, 3, 2, 5)).astype(ml_dtypes.bfloat16)
        in_maps.append({
            "xt": xt,
            "xh_d": xh_d,
            "wqkv": _wtile(wqkv_r).astype(ml_dtypes.bfloat16),
            "alibiT": alE,
            "wproj": np.ascontiguousarray(
                w_proj[r * CL:(r + 1) * CL, :]).astype(ml_dtypes.bfloat16),
            "wfc1": _wtile(wf1[:, r * DL:(r + 1) * DL]
                           ).astype(ml_dtypes.bfloat16),
            "wfc2": _wtile(wf2[:, r * DL:(r + 1) * DL]
                           ).astype(ml_dtypes.bfloat16),
            "wmp": np.ascontiguousarray(
                w_mlp_proj[r * DL:(r + 1) * DL, :]).astype(ml_dtypes.bfloat16),
        })
    return in_maps


def assemble(results):
    full = np.empty((T, C), dtype=np.float32)
    for r in range(NCORES):
        o = results[r]["out"].astype(np.float32)   # [NT4, 256, 512]
        full[:, r * (C // NCORES):(r + 1) * (C // NCORES)] = \
            o.transpose(0, 2, 1).reshape(T, C // NCORES)
    return full[None, :, :]


def kernel(x, alibi, w_attn, w_proj, w_fc1, w_fc2, w_mlp_proj,
           rms1_scale, rms2_scale):
    nc = _get_nc()
    in_maps = make_in_maps(x, alibi, w_attn, w_proj, w_fc1, w_fc2, w_mlp_proj,
                           rms1_scale, rms2_scale)
    res = run_bass_kernel_spmd(nc, in_maps, core_ids=list(range(NCORES)))
    return assemble(res.results)
